# revision 9
# baseline (speedup 1.0000x reference)
"""Locally-connected graph-conv kernel for Trainium2 (Bass/Tile).

Computes out[b,t,m] = sum_n x[b,t,n] * (S*W)[n,m] + bias[m] for
x [64, 2048, 208], W/S [208, 208], bias [208].

The ring-graph support S is a +-4 band (mod 208): output node m only
depends on x nodes m-4..m+4. The 208 outputs are split into FOUR
groups of 52, each needing a 60-row contraction slice, and the four
[60,52] premasked weight tiles are packed into the 2x2 quadrants of
the 128x128 PE array via tile_position:
    G0 outs   0.. 51  rot rows   0.. 59  quadrant (0,0)    x-tile E
    G1 outs  52..103  rot rows  52..111  quadrant (64,64)  x-tile E
    G2 outs 104..155  rot rows 104..163  quadrant (0,64)   x-tile O
    G3 outs 156..207  rot rows 156..215  quadrant (64,0)   x-tile O
(rot row j = node (j-4) mod 208). Per 512 t-columns, FOUR matmuls run
CONCURRENTLY in the four quadrants (hardware per-subarray concurrency;
LDWEIGHTS for one quadrant overlaps in-flight matmuls in others), so
every t-column is streamed once per x-tile instead of once per
104-output block: ~2x the PE throughput of the 2-block layout and,
critically, fast enough (~2.2 us per 2048 cols even at the cold 1.2
GHz HAM clock) that the PE never paces the DMA pipeline - no HAM
warm-up games needed.

The host stacks x into the two quadrant layouts (E: G0 rows at
partitions 0:60, G1 at 64:124; O: G2/G3 likewise; 4 junk rows pad each
60-row group to the 64-partition quadrant boundary). G0/G1 land in the
same PSUM bank at partitions 0:52 and 64:116, so one [116,x] eviction
per bank-pair moves both (junk partitions 52:64 ride along and are
dropped by the host at gather).

Everything that touches HBM is bf16 (PSUM accumulation stays fp32).
Measured HW behavior this build is tuned against:
 - ONE HWDGE ring sustains only ~260 GB/s; the ~360 GB/s HBM rate
   needs both rings pulling. Loads split stream-wise: xE chunks on the
   Sync ring (wh at its head), xO chunks on the Scalar ring (bias at
   its head), strictly front-to-back so the head chunk is never
   starved behind later bytes. Stores INTERLEAVE with the loads on the
   same two rings (oA after each vector eviction on Sync, oB after
   each scalar eviction on Scalar): measured, the GpSimd SWDGE ring
   moves the ragged 116-partition stores at only ~141 GB/s (sub-KB
   packets) and its backlog backpressures the whole pipeline, while
   the HWDGE rings treat an interleaved store as just more FIFO bytes.
 - Each engine issues the chunk-c store right after its own chunk-c
   eviction, then the chunk c+PREF load: load issue stays PREF chunks
   ahead of consumption and the ring FIFO carries bytes in the order
   the pipeline needs them.
 - chunk sizes taper at BOTH ends: small first chunks start compute
   early, small last chunks keep the store tail short.
 - PSUM->SBUF eviction is 1 elem/lane/cycle (fp32 source), so the E
   bank-pair evicts on VectorE and the O bank-pair on ScalarE, both
   fusing bias and the fp32->bf16 down-convert.
The host transposes y^T back at gather.
"""

import numpy as np
import ml_dtypes
from contextlib import ExitStack

import concourse.bacc as bacc
import concourse.mybir as mybir
import concourse.tile as tile
from concourse.bass_utils import run_bass_kernel_spmd

N = 208                      # nodes
K = 4                        # band half-width of S
G = 52                       # output nodes per PE quadrant tile
GR = G + 2 * K               # 60 contraction rows per group
QP = 64                      # quadrant partition pitch
GE = QP + GR                 # 124 used partitions per x tile
EV = QP + G                  # 116 evicted partitions per bank-pair
XROWS = 128                  # x tile partition count (DMA-friendly)
WPAD = 1024                  # wh DRAM row padding (2 KB rows -> fast DMA)
BPAD = 256                   # bias DRAM row padding (1 KB f32 rows)
N_CORES = 8
B, T = 64, 2048
ROWS_TOTAL = B * T           # 131072
SHARD = ROWS_TOTAL // N_CORES    # 16384 rows per core
TB = 512                     # moving-block columns per matmul (fp32 PSUM max)
TB2 = 2 * TB                 # eviction group (2 PSUM banks)
CHUNKS = [1024, 1024, 2048, 2048, 2048, 2048, 2048, 1024, 1024, 1024, 1024]
assert sum(CHUNKS) == SHARD
PREF = 4                     # chunks of load-issue lookahead per ring

FP32 = mybir.dt.float32
BF16 = mybir.dt.bfloat16
NP_BF16 = ml_dtypes.bfloat16
IDENT = mybir.ActivationFunctionType.Identity

_CACHE = {}
LAST_RESULTS = None          # BassKernelResults of the most recent run


def _kernel_body(tc):
    nc = tc.nc
    x_e = nc.dram_tensor("xe", [XROWS, SHARD], BF16, kind="ExternalInput").ap()
    x_o = nc.dram_tensor("xo", [XROWS, SHARD], BF16, kind="ExternalInput").ap()
    w_d = nc.dram_tensor("wh", [XROWS, WPAD], BF16, kind="ExternalInput").ap()
    b_d = nc.dram_tensor("bias", [XROWS, BPAD], FP32, kind="ExternalInput").ap()
    o_d = nc.dram_tensor("outt", [2 * EV, SHARD], BF16, kind="ExternalOutput").ap()

    with ExitStack() as ctx:
        const = ctx.enter_context(tc.tile_pool(name="const", bufs=1))

        # Ring heads: wh leads Sync, bias leads Scalar (both tiny, done
        # in <1 us at the head of their FIFOs).
        wh = const.tile([XROWS, WPAD], BF16, tag="wh")
        nc.sync.dma_start(wh, w_d)
        bt = const.tile([XROWS, BPAD], FP32, tag="bt")
        nc.scalar.dma_start(bt, b_d)
        bAc = bt[0:EV, 0:1]
        bBc = bt[0:EV, 1:2]

        oAp = ctx.enter_context(tc.tile_pool(name="oAp", bufs=3))
        oBp = ctx.enter_context(tc.tile_pool(name="oBp", bufs=3))
        psAp = ctx.enter_context(tc.tile_pool(name="psAp", bufs=2, space="PSUM"))
        psBp = ctx.enter_context(tc.tile_pool(name="psBp", bufs=2, space="PSUM"))

        # persistent x tiles; loads issued chunk-order, xE on Sync.
        # xO on Scalar, interleaved with evictions below (PREF ahead).
        xts = []
        col = 0
        for c, csz in enumerate(CHUNKS):
            xe = const.tile([XROWS, csz], BF16, tag=f"xe_{c}")
            xo = const.tile([XROWS, csz], BF16, tag=f"xo_{c}")
            xts.append((xe, xo, col, csz))
            col += csz

        def issue_loads(c):
            xe, xo, col, csz = xts[c]
            lsl = slice(col, col + csz)
            nc.sync.dma_start(xe, x_e[:, lsl])
            nc.scalar.dma_start(xo, x_o[:, lsl])

        for c in range(PREF):
            issue_loads(c)

        n_chunks = len(CHUNKS)
        for c, (xe, xo, col, csz) in enumerate(xts):
            tsl = slice(col, col + csz)
            oA_t = oAp.tile([EV, csz], BF16, tag="oA")
            oB_t = oBp.tile([EV, csz], BF16, tag="oB")
            for s in range((csz + TB2 - 1) // TB2):
                g0 = s * TB2
                gw = min(TB2, csz - g0)
                g = slice(g0, g0 + gw)
                # [128, 1024] PSUM tiles (2 banks); each of the four
                # quadrant matmuls targets one bank, partitions 0:52 or
                # 64:116.
                psA = psAp.tile([XROWS, TB2], FP32, tag="psA")
                psB = psBp.tile([XROWS, TB2], FP32, tag="psB")
                for q0 in range(0, gw, TB):
                    qs = slice(g0 + q0, g0 + q0 + TB)
                    qp = slice(q0, q0 + TB)
                    nc.tensor.matmul(psA[0:G, qp], wh[0:GR, 0:G],
                                     xe[0:GR, qs], start=True, stop=True,
                                     tile_position=(0, 0))
                    nc.tensor.matmul(psA[QP:EV, qp], wh[QP:GE, 0:G],
                                     xe[QP:GE, qs], start=True, stop=True,
                                     tile_position=(QP, QP))
                    nc.tensor.matmul(psB[QP:EV, qp], wh[0:GR, G : 2 * G],
                                     xo[0:GR, qs], start=True, stop=True,
                                     tile_position=(0, QP))
                    nc.tensor.matmul(psB[0:G, qp], wh[QP:GE, G : 2 * G],
                                     xo[QP:GE, qs], start=True, stop=True,
                                     tile_position=(QP, 0))
                # one [116,gw] eviction per bank-pair moves both groups
                # (junk partitions 52:64 ride along); bias + fp32->bf16
                # fused. E pair on VectorE, O pair on ScalarE.
                nc.vector.tensor_scalar_add(oA_t[:, g], psA[0:EV, 0:gw], bAc)
                nc.scalar.activation(oB_t[:, g], psB[0:EV, 0:gw], IDENT, bias=bBc)
            # stores interleave with loads on the HWDGE rings: oA on
            # Sync after the vector eviction, oB on Scalar after the
            # scalar eviction; then each engine issues its chunk
            # c+PREF load so issue stays PREF chunks ahead.
            nc.sync.dma_start(o_d[0:EV, tsl], oA_t)
            nc.scalar.dma_start(o_d[EV : 2 * EV, tsl], oB_t)
            if c + PREF < n_chunks:
                issue_loads(c + PREF)


def _build():
    nc = bacc.Bacc(
        "TRN2",
        target_bir_lowering=False,
        debug=False,
        num_devices=N_CORES,
    )
    with tile.TileContext(nc) as tc:
        _kernel_body(tc)
    nc.compile()
    return nc


def kernel(x, W, b, S):
    global LAST_RESULTS
    nc = _CACHE.get("nc")
    if nc is None:
        nc = _build()
        _CACHE["nc"] = nc

    xf = np.asarray(x, np.float32).reshape(ROWS_TOTAL, N)
    SW = (np.asarray(S, np.float32) * np.asarray(W, np.float32))
    rot = [(r - K) % N for r in range(N + 2 * K)]       # rot row -> node
    SWr = SW[rot, :]                                    # [216, 208]
    wh = np.zeros((XROWS, WPAD), NP_BF16)
    wh[0:GR, 0:G] = SWr[0:GR, 0:G]                      # G0
    wh[QP:GE, 0:G] = SWr[G : G + GR, G : 2 * G]         # G1
    wh[0:GR, G : 2 * G] = SWr[2 * G : 2 * G + GR, 2 * G : 3 * G]   # G2
    wh[QP:GE, G : 2 * G] = SWr[3 * G : 3 * G + GR, 3 * G : 4 * G]  # G3
    bfv = np.asarray(b, np.float32).reshape(N)
    bf = np.zeros((XROWS, BPAD), np.float32)
    bf[0:G, 0] = bfv[0:G]                # E pair col 0: G0 at 0:52
    bf[QP:EV, 0] = bfv[G : 2 * G]        #               G1 at 64:116
    bf[0:G, 1] = bfv[3 * G : 4 * G]      # O pair col 1: G3 at 0:52
    bf[QP:EV, 1] = bfv[2 * G : 3 * G]    #               G2 at 64:116

    in_maps = []
    for i in range(N_CORES):
        xt = xf[i * SHARD : (i + 1) * SHARD].T          # [208, SHARD] view
        xr = np.empty((N + 2 * K, SHARD), NP_BF16)      # rotated rows
        xr[0:K] = xt[N - K : N]
        xr[K : N + K] = xt
        xr[N + K :] = xt[0:K]
        xe = np.zeros((XROWS, SHARD), NP_BF16)
        xe[0:GR] = xr[0:GR]                             # G0 rows
        xe[QP:GE] = xr[G : G + GR]                      # G1 rows
        xo = np.zeros((XROWS, SHARD), NP_BF16)
        xo[0:GR] = xr[2 * G : 2 * G + GR]               # G2 rows
        xo[QP:GE] = xr[3 * G : 3 * G + GR]              # G3 rows
        in_maps.append({"xe": xe, "xo": xo, "wh": wh, "bias": bf})
    res = run_bass_kernel_spmd(nc, in_maps, core_ids=list(range(N_CORES)))
    LAST_RESULTS = res
    out = np.empty((ROWS_TOTAL, N), np.float32)
    for i, r in enumerate(res.results):
        yt = r["outt"]                                  # [232, SHARD] bf16
        sl = slice(i * SHARD, (i + 1) * SHARD)
        out[sl, 0:G] = yt[0:G].T                        # G0
        out[sl, G : 2 * G] = yt[QP:EV].T                # G1
        out[sl, 3 * G : 4 * G] = yt[EV : EV + G].T      # G3
        out[sl, 2 * G : 3 * G] = yt[EV + QP : 2 * EV].T # G2
    return out.reshape(B, T, N)


# revision 11
# speedup vs baseline: 1.7195x; 1.7195x over previous
"""Locally-connected graph-conv kernel for Trainium2 (Bass/Tile).

Computes out[b,t,m] = sum_n x[b,t,n] * (S*W)[n,m] + bias[m] for
x [64, 2048, 208], W/S [208, 208], bias [208].

The ring-graph support S is a +-4 band (mod 208): output node m only
depends on x nodes m-4..m+4. The 208 outputs are split into FOUR
groups of 52, each needing a 60-row contraction slice, and the four
[60,52] premasked weight tiles are packed into the 2x2 quadrants of
the 128x128 PE array via tile_position:
    G0 outs   0.. 51  rot rows   0.. 59  quadrant (0,0)    x-tile E
    G1 outs  52..103  rot rows  52..111  quadrant (64,64)  x-tile E
    G2 outs 104..155  rot rows 104..163  quadrant (0,64)   x-tile O
    G3 outs 156..207  rot rows 156..215  quadrant (64,0)   x-tile O
(rot row j = node (j-4) mod 208). Per 512 t-columns, FOUR matmuls run
CONCURRENTLY in the four quadrants (hardware per-subarray concurrency;
LDWEIGHTS for one quadrant overlaps in-flight matmuls in others), so
every t-column is streamed once per x-tile instead of once per
104-output block: ~2x the PE throughput of the 2-block layout and fast
enough (~2.2 us per 2048 cols even at the cold HAM clock) that the PE
never paces the DMA pipeline - no HAM warm-up games needed.

Quadrant alignment wants G0/G2 rows at SBUF partitions 0:60 and G1/G3
at 64:124 (matmul moving-operand base partition must equal the
tile_position row), and results land at PSUM partitions 0:52 and
64:116. All HBM transfers use plain full-128-partition shapes - the
host pads the x streams to 128 rows and unpacks the out streams from
rows {0:52, 64:116} - because (measured) two-level partition APs
silently corrupt DMA addressing and ragged partition counts fall off
the DMA fast path (116-row SWDGE stores: 141 GB/s vs 220+ at 128).

Everything that touches HBM is bf16 (PSUM accumulation stays fp32).
Measured HW behavior this build is tuned against:
 - ONE HWDGE ring sustains only ~260 GB/s, and a DMA issue that waits
   on a compute semaphore BLOCKS the issuing engine - interleaving
   stores into a load ring collapses it to ~100 GB/s. So: ALL load
   issues go up-front (Sync: wh then xE chunks; Scalar: bias then xO
   chunks; engines never block while their ring still has load bytes),
   oA stores ride the GpSimd SWDGE ring (~220 GB/s on clean shapes),
   and oB stores alternate between the Scalar and Sync rings AFTER all
   load issues (blocking on eviction sems is then harmless), balancing
   the three rings at ~6.6/6.4/4.2 MB.
 - chunk sizes taper at BOTH ends: small first chunks start compute
   early, small last chunks keep the store tail short.
 - PSUM->SBUF eviction is 1 elem/lane/cycle (fp32 source): the E
   bank-pair evicts on VectorE, the O bank-pair on ScalarE, one
   [116,x] op per bank-pair, fusing bias and the fp32->bf16 convert.
The host transposes y^T back at gather.
"""

import numpy as np
import ml_dtypes
from contextlib import ExitStack

import concourse.bacc as bacc
import concourse.mybir as mybir
import concourse.tile as tile
from concourse.bass_utils import run_bass_kernel_spmd

N = 208                      # nodes
K = 4                        # band half-width of S
G = 52                       # output nodes per PE quadrant tile
GR = G + 2 * K               # 60 contraction rows per group
QP = 64                      # quadrant partition pitch
GE = QP + GR                 # 124 used partitions per x tile
EV = QP + G                  # 116 evicted partitions per bank-pair
XROWS = 128                  # x/out tile + DRAM stream partition count
WPAD = 1024                  # wh DRAM row padding (2 KB rows -> fast DMA)
BPAD = 256                   # bias DRAM row padding (1 KB f32 rows)
N_CORES = 8
B, T = 64, 2048
ROWS_TOTAL = B * T           # 131072
SHARD = ROWS_TOTAL // N_CORES    # 16384 rows per core
TB = 512                     # moving-block columns per matmul (fp32 PSUM max)
TB2 = 2 * TB                 # eviction group (2 PSUM banks)
CHUNKS = [1024, 1024, 2048, 2048, 2048, 2048, 2048, 1024, 1024, 1024, 1024]
assert sum(CHUNKS) == SHARD

FP32 = mybir.dt.float32
BF16 = mybir.dt.bfloat16
NP_BF16 = ml_dtypes.bfloat16
IDENT = mybir.ActivationFunctionType.Identity

_CACHE = {}
LAST_RESULTS = None          # BassKernelResults of the most recent run


def _kernel_body(tc):
    nc = tc.nc
    x_e = nc.dram_tensor("xe", [XROWS, SHARD], BF16, kind="ExternalInput").ap()
    x_o = nc.dram_tensor("xo", [XROWS, SHARD], BF16, kind="ExternalInput").ap()
    w_d = nc.dram_tensor("wh", [XROWS, WPAD], BF16, kind="ExternalInput").ap()
    b_d = nc.dram_tensor("bias", [XROWS, BPAD], FP32, kind="ExternalInput").ap()
    o_d = nc.dram_tensor("outt", [2 * XROWS, SHARD], BF16, kind="ExternalOutput").ap()

    with ExitStack() as ctx:
        const = ctx.enter_context(tc.tile_pool(name="const", bufs=1))

        # Ring heads: wh leads Sync, bias leads Scalar (both tiny, done
        # in <1 us at the head of their FIFOs).
        wh = const.tile([XROWS, WPAD], BF16, tag="wh")
        nc.sync.dma_start(wh, w_d)
        bt = const.tile([XROWS, BPAD], FP32, tag="bt")
        nc.scalar.dma_start(bt, b_d)
        bAc = bt[0:EV, 0:1]
        bBc = bt[0:EV, 1:2]

        oAp = ctx.enter_context(tc.tile_pool(name="oAp", bufs=3))
        oBp = ctx.enter_context(tc.tile_pool(name="oBp", bufs=3))
        psAp = ctx.enter_context(tc.tile_pool(name="psAp", bufs=2, space="PSUM"))
        psBp = ctx.enter_context(tc.tile_pool(name="psBp", bufs=2, space="PSUM"))

        # ALL load issues up-front, chunk-order, so neither HWDGE
        # engine ever blocks on a compute semaphore while its ring has
        # load bytes left to move.
        xts = []
        col = 0
        for c, csz in enumerate(CHUNKS):
            xe = const.tile([XROWS, csz], BF16, tag=f"xe_{c}")
            xo = const.tile([XROWS, csz], BF16, tag=f"xo_{c}")
            xts.append((xe, xo, col, csz))
            col += csz
        for xe, xo, col, csz in xts:
            nc.sync.dma_start(xe, x_e[:, col : col + csz])
        for xe, xo, col, csz in xts:
            nc.scalar.dma_start(xo, x_o[:, col : col + csz])

        for c, (xe, xo, col, csz) in enumerate(xts):
            tsl = slice(col, col + csz)
            oA_t = oAp.tile([XROWS, csz], BF16, tag="oA")
            oB_t = oBp.tile([XROWS, csz], BF16, tag="oB")
            for s in range((csz + TB2 - 1) // TB2):
                g0 = s * TB2
                gw = min(TB2, csz - g0)
                g = slice(g0, g0 + gw)
                # [128, 1024] PSUM tiles (2 banks); each of the four
                # quadrant matmuls targets one bank, partitions 0:52 /
                # 64:116.
                psA = psAp.tile([XROWS, TB2], FP32, tag="psA")
                psB = psBp.tile([XROWS, TB2], FP32, tag="psB")
                for q0 in range(0, gw, TB):
                    qs = slice(g0 + q0, g0 + q0 + TB)
                    qp = slice(q0, q0 + TB)
                    nc.tensor.matmul(psA[0:G, qp], wh[0:GR, 0:G],
                                     xe[0:GR, qs], start=True, stop=True,
                                     tile_position=(0, 0))
                    nc.tensor.matmul(psA[QP:EV, qp], wh[QP:GE, 0:G],
                                     xe[QP:GE, qs], start=True, stop=True,
                                     tile_position=(QP, QP))
                    nc.tensor.matmul(psB[QP:EV, qp], wh[0:GR, G : 2 * G],
                                     xo[0:GR, qs], start=True, stop=True,
                                     tile_position=(0, QP))
                    nc.tensor.matmul(psB[0:G, qp], wh[QP:GE, G : 2 * G],
                                     xo[QP:GE, qs], start=True, stop=True,
                                     tile_position=(QP, 0))
                # one [116,gw] eviction per bank-pair moves both groups
                # (junk partitions 52:64 ride along); bias + fp32->bf16
                # fused. E pair on VectorE, O pair on ScalarE.
                nc.vector.tensor_scalar_add(oA_t[0:EV, g], psA[0:EV, 0:gw], bAc)
                nc.scalar.activation(oB_t[0:EV, g], psB[0:EV, 0:gw], IDENT, bias=bBc)
            # stores: oA on the GpSimd SWDGE ring; oB alternates Scalar
            # / Sync (their engines are done issuing loads; blocking on
            # the eviction sem no longer starves a load stream).
            nc.gpsimd.dma_start(o_d[0:XROWS, tsl], oA_t)
            oB_eng = nc.scalar if c % 2 == 0 else nc.sync
            oB_eng.dma_start(o_d[XROWS : 2 * XROWS, tsl], oB_t)


def _build():
    nc = bacc.Bacc(
        "TRN2",
        target_bir_lowering=False,
        debug=False,
        num_devices=N_CORES,
    )
    with tile.TileContext(nc) as tc:
        _kernel_body(tc)
    nc.compile()
    return nc


def kernel(x, W, b, S):
    global LAST_RESULTS
    nc = _CACHE.get("nc")
    if nc is None:
        nc = _build()
        _CACHE["nc"] = nc

    xf = np.asarray(x, np.float32).reshape(ROWS_TOTAL, N)
    SW = (np.asarray(S, np.float32) * np.asarray(W, np.float32))
    rot = [(r - K) % N for r in range(N + 2 * K)]       # rot row -> node
    SWr = SW[rot, :]                                    # [216, 208]
    wh = np.zeros((XROWS, WPAD), NP_BF16)
    wh[0:GR, 0:G] = SWr[0:GR, 0:G]                      # G0
    wh[QP:GE, 0:G] = SWr[G : G + GR, G : 2 * G]         # G1
    wh[0:GR, G : 2 * G] = SWr[2 * G : 2 * G + GR, 2 * G : 3 * G]   # G2
    wh[QP:GE, G : 2 * G] = SWr[3 * G : 3 * G + GR, 3 * G : 4 * G]  # G3
    bfv = np.asarray(b, np.float32).reshape(N)
    bf = np.zeros((XROWS, BPAD), np.float32)
    bf[0:G, 0] = bfv[0:G]                # E pair col 0: G0 at 0:52
    bf[QP:EV, 0] = bfv[G : 2 * G]        #               G1 at 64:116
    bf[0:G, 1] = bfv[3 * G : 4 * G]      # O pair col 1: G3 at 0:52
    bf[QP:EV, 1] = bfv[2 * G : 3 * G]    #               G2 at 64:116

    in_maps = []
    for i in range(N_CORES):
        xt = xf[i * SHARD : (i + 1) * SHARD].T          # [208, SHARD] view
        xr = np.empty((N + 2 * K, SHARD), NP_BF16)      # rotated rows
        xr[0:K] = xt[N - K : N]
        xr[K : N + K] = xt
        xr[N + K :] = xt[0:K]
        xe = np.zeros((XROWS, SHARD), NP_BF16)
        xe[0:GR] = xr[0:GR]                             # G0 rows
        xe[QP:GE] = xr[G : G + GR]                      # G1 rows
        xo = np.zeros((XROWS, SHARD), NP_BF16)
        xo[0:GR] = xr[2 * G : 2 * G + GR]               # G2 rows
        xo[QP:GE] = xr[3 * G : 3 * G + GR]              # G3 rows
        in_maps.append({"xe": xe, "xo": xo, "wh": wh, "bias": bf})
    res = run_bass_kernel_spmd(nc, in_maps, core_ids=list(range(N_CORES)))
    LAST_RESULTS = res
    out = np.empty((ROWS_TOTAL, N), np.float32)
    for i, r in enumerate(res.results):
        yt = r["outt"]                                  # [256, SHARD] bf16
        sl = slice(i * SHARD, (i + 1) * SHARD)
        out[sl, 0:G] = yt[0:G].T                        # G0
        out[sl, G : 2 * G] = yt[QP:EV].T                # G1
        out[sl, 3 * G : 4 * G] = yt[XROWS : XROWS + G].T        # G3
        out[sl, 2 * G : 3 * G] = yt[XROWS + QP : XROWS + EV].T  # G2
    return out.reshape(B, T, N)


# revision 15
# speedup vs baseline: 1.8025x; 1.0483x over previous
"""Locally-connected graph-conv kernel for Trainium2 (Bass/Tile).

Computes out[b,t,m] = sum_n x[b,t,n] * (S*W)[n,m] + bias[m] for
x [64, 2048, 208], W/S [208, 208], bias [208].

The ring-graph support S is a +-4 band (mod 208): output node m only
depends on x nodes m-4..m+4. The 208 outputs are split into FOUR
groups of 52, each needing a 60-row contraction slice, and the four
[60,52] premasked weight tiles are packed into the 2x2 quadrants of
the 128x128 PE array via tile_position:
    G0 outs   0.. 51  rot rows   0.. 59  quadrant (0,0)    x-tile E
    G1 outs  52..103  rot rows  52..111  quadrant (64,64)  x-tile E
    G2 outs 104..155  rot rows 104..163  quadrant (0,64)   x-tile O
    G3 outs 156..207  rot rows 156..215  quadrant (64,0)   x-tile O
(rot row j = node (j-4) mod 208). Per 512 t-columns, FOUR matmuls run
CONCURRENTLY in the four quadrants (hardware per-subarray concurrency;
LDWEIGHTS for one quadrant overlaps in-flight matmuls in others), so
every t-column is streamed once per x-tile instead of once per
104-output block: ~2x the PE throughput of the 2-block layout and fast
enough (~2.2 us per 2048 cols even at the cold HAM clock) that the PE
never paces the DMA pipeline - no HAM warm-up games needed.

Quadrant alignment wants G0/G2 rows at SBUF partitions 0:60 and G1/G3
at 64:124 (matmul moving-operand base partition must equal the
tile_position row), and results land at PSUM partitions 0:52 and
64:116. All HBM transfers use plain full-128-partition shapes - the
host pads the x streams to 128 rows and unpacks the out streams from
rows {0:52, 64:116} - because (measured) two-level partition APs
silently corrupt DMA addressing and ragged partition counts fall off
the DMA fast path (116-row SWDGE stores: 141 GB/s vs 220+ at 128).

Everything that touches HBM is bf16 (PSUM accumulation stays fp32).
Measured HW behavior this build is tuned against:
 - ONE HWDGE ring sustains only ~260 GB/s; mixed read+write FIFO
   interleave on BOTH rings is the measured ~400+ GB/s regime. A DMA
   issue that waits on a compute semaphore BLOCKS the issuing engine,
   so load issues run PREF chunks ahead of the store issues (Sync: wh,
   xE; Scalar: bias, xO; stores oA/oB follow their evictions): each
   ring FIFO alternates [store c][load c+PREF] and never sits empty
   while an engine waits on an eviction sem. Ragged partition counts
   must be avoided (116-row HWDGE stores measured 99 GB/s ring-wide) -
   every transfer here is a full 128 partitions.
 - chunk sizes taper at BOTH ends: small first chunks start compute
   early, small last chunks keep the store tail short.
 - PSUM->SBUF eviction is 1 elem/lane/cycle (fp32 source): the E
   bank-pair evicts on VectorE, the O bank-pair on ScalarE, one
   [116,x] op per bank-pair, fusing bias and the fp32->bf16 convert.
The host transposes y^T back at gather.
"""

import numpy as np
import ml_dtypes
from contextlib import ExitStack

import concourse.bacc as bacc
import concourse.mybir as mybir
import concourse.tile as tile
from concourse.bass_utils import run_bass_kernel_spmd

N = 208                      # nodes
K = 4                        # band half-width of S
G = 52                       # output nodes per PE quadrant tile
GR = G + 2 * K               # 60 contraction rows per group
QP = 64                      # quadrant partition pitch
GE = QP + GR                 # 124 used partitions per x tile
EV = QP + G                  # 116 evicted partitions per bank-pair
XROWS = 128                  # x/out tile + DRAM stream partition count
WPAD = 1024                  # wh DRAM row padding (2 KB rows -> fast DMA)
BPAD = 256                   # bias DRAM row padding (1 KB f32 rows)
N_CORES = 8
B, T = 64, 2048
ROWS_TOTAL = B * T           # 131072
SHARD = ROWS_TOTAL // N_CORES    # 16384 rows per core
TB = 512                     # moving-block columns per matmul (fp32 PSUM max)
TB2 = 2 * TB                 # eviction group (2 PSUM banks)
CHUNKS = [1024, 1024, 2048, 2048, 2048, 2048, 2048, 1024, 1024, 1024, 1024]
assert sum(CHUNKS) == SHARD
PREF = 4                     # chunks of load-issue lookahead per ring

FP32 = mybir.dt.float32
BF16 = mybir.dt.bfloat16
NP_BF16 = ml_dtypes.bfloat16
IDENT = mybir.ActivationFunctionType.Identity

_CACHE = {}
LAST_RESULTS = None          # BassKernelResults of the most recent run


def _kernel_body(tc):
    nc = tc.nc
    x_e = nc.dram_tensor("xe", [XROWS, SHARD], BF16, kind="ExternalInput").ap()
    x_o = nc.dram_tensor("xo", [XROWS, SHARD], BF16, kind="ExternalInput").ap()
    w_d = nc.dram_tensor("wh", [XROWS, WPAD], BF16, kind="ExternalInput").ap()
    b_d = nc.dram_tensor("bias", [XROWS, BPAD], FP32, kind="ExternalInput").ap()
    o_d = nc.dram_tensor("outt", [2 * XROWS, SHARD], BF16, kind="ExternalOutput").ap()

    with ExitStack() as ctx:
        const = ctx.enter_context(tc.tile_pool(name="const", bufs=1))

        # Ring heads: wh leads Sync, bias leads Scalar (both tiny, done
        # in <1 us at the head of their FIFOs).
        wh = const.tile([XROWS, WPAD], BF16, tag="wh")
        nc.sync.dma_start(wh, w_d)
        bt = const.tile([XROWS, BPAD], FP32, tag="bt")
        nc.scalar.dma_start(bt, b_d)
        bAc = bt[0:EV, 0:1]
        bBc = bt[0:EV, 1:2]

        oAp = ctx.enter_context(tc.tile_pool(name="oAp", bufs=3))
        oBp = ctx.enter_context(tc.tile_pool(name="oBp", bufs=3))
        psAp = ctx.enter_context(tc.tile_pool(name="psAp", bufs=2, space="PSUM"))
        psBp = ctx.enter_context(tc.tile_pool(name="psBp", bufs=2, space="PSUM"))

        # Load issues run PREF chunks ahead of the store issues below,
        # so each HWDGE ring FIFO alternates [store c][load c+PREF]
        # bytes: reads and writes interleave on both rings (the regime
        # measured at ~400+ GB/s combined) and a store's eviction-sem
        # wait never leaves its ring without queued load bytes.
        xts = []
        col = 0
        for c, csz in enumerate(CHUNKS):
            xe = const.tile([XROWS, csz], BF16, tag=f"xe_{c}")
            xo = const.tile([XROWS, csz], BF16, tag=f"xo_{c}")
            xts.append((xe, xo, col, csz))
            col += csz

        def issue_loads(c):
            xe, xo, col, csz = xts[c]
            nc.sync.dma_start(xe, x_e[:, col : col + csz])
            nc.scalar.dma_start(xo, x_o[:, col : col + csz])

        for c in range(PREF):
            issue_loads(c)

        for c, (xe, xo, col, csz) in enumerate(xts):
            tsl = slice(col, col + csz)
            oA_t = oAp.tile([XROWS, csz], BF16, tag="oA")
            oB_t = oBp.tile([XROWS, csz], BF16, tag="oB")
            for s in range((csz + TB2 - 1) // TB2):
                g0 = s * TB2
                gw = min(TB2, csz - g0)
                g = slice(g0, g0 + gw)
                # [128, 1024] PSUM tiles (2 banks); each of the four
                # quadrant matmuls targets one bank, partitions 0:52 /
                # 64:116.
                psA = psAp.tile([XROWS, TB2], FP32, tag="psA")
                psB = psBp.tile([XROWS, TB2], FP32, tag="psB")
                for q0 in range(0, gw, TB):
                    qs = slice(g0 + q0, g0 + q0 + TB)
                    qp = slice(q0, q0 + TB)
                    nc.tensor.matmul(psA[0:G, qp], wh[0:GR, 0:G],
                                     xe[0:GR, qs], start=True, stop=True,
                                     tile_position=(0, 0))
                    nc.tensor.matmul(psA[QP:EV, qp], wh[QP:GE, 0:G],
                                     xe[QP:GE, qs], start=True, stop=True,
                                     tile_position=(QP, QP))
                    nc.tensor.matmul(psB[QP:EV, qp], wh[0:GR, G : 2 * G],
                                     xo[0:GR, qs], start=True, stop=True,
                                     tile_position=(0, QP))
                    nc.tensor.matmul(psB[0:G, qp], wh[QP:GE, G : 2 * G],
                                     xo[QP:GE, qs], start=True, stop=True,
                                     tile_position=(QP, 0))
                # one [116,gw] eviction per bank-pair moves both groups
                # (junk partitions 52:64 ride along); bias + fp32->bf16
                # fused. E pair on VectorE, O pair on ScalarE.
                nc.vector.tensor_scalar_add(oA_t[0:EV, g], psA[0:EV, 0:gw], bAc)
                nc.scalar.activation(oB_t[0:EV, g], psB[0:EV, 0:gw], IDENT, bias=bBc)
            # stores interleave with loads on the two HWDGE rings: oA
            # on Sync (after the vector eviction), oB on Scalar (after
            # the scalar eviction); then each engine issues its chunk
            # c+PREF loads.
            nc.sync.dma_start(o_d[0:XROWS, tsl], oA_t)
            nc.scalar.dma_start(o_d[XROWS : 2 * XROWS, tsl], oB_t)
            if c + PREF < len(CHUNKS):
                issue_loads(c + PREF)


def _build():
    nc = bacc.Bacc(
        "TRN2",
        target_bir_lowering=False,
        debug=False,
        num_devices=N_CORES,
    )
    with tile.TileContext(nc) as tc:
        _kernel_body(tc)
    nc.compile()
    return nc


def kernel(x, W, b, S):
    global LAST_RESULTS
    nc = _CACHE.get("nc")
    if nc is None:
        nc = _build()
        _CACHE["nc"] = nc

    xf = np.asarray(x, np.float32).reshape(ROWS_TOTAL, N)
    SW = (np.asarray(S, np.float32) * np.asarray(W, np.float32))
    rot = [(r - K) % N for r in range(N + 2 * K)]       # rot row -> node
    SWr = SW[rot, :]                                    # [216, 208]
    wh = np.zeros((XROWS, WPAD), NP_BF16)
    wh[0:GR, 0:G] = SWr[0:GR, 0:G]                      # G0
    wh[QP:GE, 0:G] = SWr[G : G + GR, G : 2 * G]         # G1
    wh[0:GR, G : 2 * G] = SWr[2 * G : 2 * G + GR, 2 * G : 3 * G]   # G2
    wh[QP:GE, G : 2 * G] = SWr[3 * G : 3 * G + GR, 3 * G : 4 * G]  # G3
    bfv = np.asarray(b, np.float32).reshape(N)
    bf = np.zeros((XROWS, BPAD), np.float32)
    bf[0:G, 0] = bfv[0:G]                # E pair col 0: G0 at 0:52
    bf[QP:EV, 0] = bfv[G : 2 * G]        #               G1 at 64:116
    bf[0:G, 1] = bfv[3 * G : 4 * G]      # O pair col 1: G3 at 0:52
    bf[QP:EV, 1] = bfv[2 * G : 3 * G]    #               G2 at 64:116

    in_maps = []
    for i in range(N_CORES):
        xt = xf[i * SHARD : (i + 1) * SHARD].T          # [208, SHARD] view
        xr = np.empty((N + 2 * K, SHARD), NP_BF16)      # rotated rows
        xr[0:K] = xt[N - K : N]
        xr[K : N + K] = xt
        xr[N + K :] = xt[0:K]
        xe = np.zeros((XROWS, SHARD), NP_BF16)
        xe[0:GR] = xr[0:GR]                             # G0 rows
        xe[QP:GE] = xr[G : G + GR]                      # G1 rows
        xo = np.zeros((XROWS, SHARD), NP_BF16)
        xo[0:GR] = xr[2 * G : 2 * G + GR]               # G2 rows
        xo[QP:GE] = xr[3 * G : 3 * G + GR]              # G3 rows
        in_maps.append({"xe": xe, "xo": xo, "wh": wh, "bias": bf})
    res = run_bass_kernel_spmd(nc, in_maps, core_ids=list(range(N_CORES)))
    LAST_RESULTS = res
    out = np.empty((ROWS_TOTAL, N), np.float32)
    for i, r in enumerate(res.results):
        yt = r["outt"]                                  # [256, SHARD] bf16
        sl = slice(i * SHARD, (i + 1) * SHARD)
        out[sl, 0:G] = yt[0:G].T                        # G0
        out[sl, G : 2 * G] = yt[QP:EV].T                # G1
        out[sl, 3 * G : 4 * G] = yt[XROWS : XROWS + G].T        # G3
        out[sl, 2 * G : 3 * G] = yt[XROWS + QP : XROWS + EV].T  # G2
    return out.reshape(B, T, N)


# revision 16
# speedup vs baseline: 1.8391x; 1.0203x over previous
"""Locally-connected graph-conv kernel for Trainium2 (Bass/Tile).

Computes out[b,t,m] = sum_n x[b,t,n] * (S*W)[n,m] + bias[m] for
x [64, 2048, 208], W/S [208, 208], bias [208].

The ring-graph support S is a +-4 band (mod 208), so each half of the
output nodes only needs a 112-row slice of the contraction dim. With a
rotated node layout (row j holds node (j-4) mod 208, 216 rows total):
  block 0 (m 0..103):   rotated rows   0..111
  block 1 (m 104..207): rotated rows 104..215
Each output block is a SINGLE [112,104] x [112,512] matmul with the
host-premasked weight block stationary in the PE array and x^T
streaming as the moving operand. This 2-block layout moves the fewest
HBM bytes of every variant tried (~15.1 MB/core all-in vs ~17.2 for
the PE-array-packed variant): at the measured ~350 GB/s effective
combined HBM rate, bytes win over PE elegance.

Everything that touches HBM is bf16 (PSUM accumulation stays fp32).
Measured HW behavior this build is tuned against:
 - Effective DMA rate: ~260 GB/s on one HWDGE ring, ~350-420 combined
   when BOTH rings carry interleaved reads+writes. Ragged (non-mult-16)
   partition counts fall off the DMA fast path (116-row stores: 99
   GB/s ring-wide) - every transfer here is 112 or 128 partitions.
 - A DMA issue that waits on a compute semaphore BLOCKS the issuing
   engine, so load issues run PREF chunks ahead of store issues (Sync:
   wh then xh0 chunks + o0 stores; Scalar: bias then xh1 chunks + o1
   stores): each ring FIFO alternates [store c][load c+PREF] and never
   sits empty while its engine waits on an eviction sem.
 - The PE runs at half clock until HAM sees ~3.4 us of sustained busy,
   and re-throttles after idle windows. 2-block matmuls are ~630 ns
   cold / ~450 warm per 512 cols incl. the per-matmul LDWEIGHTS+drain
   bubble (bass emits LDWEIGHTS per matmul unconditionally), which is
   borderline against the DMA pace - so 8 dummy matmuls on a memset
   tile (no DMA dependency, start right after the ~7 us framework
   preamble) ramp HAM before the first chunk lands, and one keep-alive
   dummy per chunk holds the activity window busy across chunk
   boundaries.
 - chunk sizes taper at BOTH ends: small first chunks start compute
   early, small last chunks keep the store tail short.
 - PSUM->SBUF eviction is 1 elem/lane/cycle (PSUM has one DVE read
   port; fp32 source), so block 0 evicts on VectorE and block 1 on
   ScalarE, both fusing bias and the fp32->bf16 down-convert.
The host transposes y^T back at gather.
"""

import numpy as np
import ml_dtypes
from contextlib import ExitStack

import concourse.bacc as bacc
import concourse.mybir as mybir
import concourse.tile as tile
from concourse.bass_utils import run_bass_kernel_spmd

N = 208                      # nodes
HALF = 104                   # output nodes per block
K = 4                        # band half-width of S
NH = 2 * K + HALF            # 112 contraction rows per block (halo incl.)
NR = N + 2 * K               # 216 rotated rows
WPAD = 1024                  # wh DRAM row padding (2 KB rows -> fast DMA)
BPAD = 256                   # bias DRAM row padding (1 KB f32 rows)
N_CORES = 8
B, T = 64, 2048
ROWS_TOTAL = B * T           # 131072
SHARD = ROWS_TOTAL // N_CORES    # 16384 rows per core
TB = 512                     # moving-block columns per matmul (fp32 PSUM max)
TB2 = 2 * TB                 # eviction group (2 PSUM banks)
CHUNKS = [1024, 1024, 2048, 2048, 2048, 2048, 2048, 1024, 1024, 1024, 1024]
assert sum(CHUNKS) == SHARD
PREF = 4                     # chunks of load-issue lookahead per ring
N_DUMMY = 8                  # PE warm-up matmuls on the memset tile

FP32 = mybir.dt.float32
BF16 = mybir.dt.bfloat16
NP_BF16 = ml_dtypes.bfloat16
IDENT = mybir.ActivationFunctionType.Identity

# halo row order (indices into the [208] node dim) for each block
ROWS0 = list(range(N - K, N)) + list(range(0, HALF + K))          # 112
ROWS1 = list(range(HALF - K, N)) + list(range(0, K))              # 112

_CACHE = {}
LAST_RESULTS = None          # BassKernelResults of the most recent run


def _kernel_body(tc):
    nc = tc.nc
    # rotated x: row j = node (j-4) mod 208; block0 = rows 0:112,
    # block1 = rows 104:216
    x_d = nc.dram_tensor("xh", [NR, SHARD], BF16, kind="ExternalInput").ap()
    w_d = nc.dram_tensor("wh", [NH, WPAD], BF16, kind="ExternalInput").ap()
    b_d = nc.dram_tensor("bias", [NH, BPAD], FP32, kind="ExternalInput").ap()
    o_d = nc.dram_tensor("outt", [2 * NH, SHARD], BF16, kind="ExternalOutput").ap()

    with ExitStack() as ctx:
        const = ctx.enter_context(tc.tile_pool(name="const", bufs=1))

        # PE warm-up fuel: memset tile, no DMA dependency, ready right
        # after the framework preamble.
        warm = const.tile([NH, TB], BF16, tag="warm")
        nc.gpsimd.memset(warm, 1.0)

        # Ring heads: wh leads Sync, bias leads Scalar (both tiny, done
        # in <1 us at the head of their FIFOs).
        wh = const.tile([NH, WPAD], BF16, tag="wh")
        nc.sync.dma_start(wh, w_d)
        bt = const.tile([NH, BPAD], FP32, tag="bt")
        nc.scalar.dma_start(bt, b_d)
        bAc = bt[0:HALF, 0:1]
        bBc = bt[0:HALF, 1:2]

        o0p = ctx.enter_context(tc.tile_pool(name="o0p", bufs=3))
        o1p = ctx.enter_context(tc.tile_pool(name="o1p", bufs=3))
        ps0p = ctx.enter_context(tc.tile_pool(name="ps0p", bufs=2, space="PSUM"))
        ps1p = ctx.enter_context(tc.tile_pool(name="ps1p", bufs=2, space="PSUM"))

        # persistent x tiles; xh0 loads on Sync, xh1 on Scalar, issued
        # PREF chunks ahead of the store issues in the main loop.
        xts = []
        col = 0
        for c, csz in enumerate(CHUNKS):
            xh0 = const.tile([NH, csz], BF16, tag=f"xh0_{c}")
            xh1 = const.tile([NH, csz], BF16, tag=f"xh1_{c}")
            xts.append((xh0, xh1, col, csz))
            col += csz

        def issue_loads(c):
            xh0, xh1, col, csz = xts[c]
            lsl = slice(col, col + csz)
            nc.sync.dma_start(xh0, x_d[0:NH, lsl])
            nc.scalar.dma_start(xh1, x_d[HALF:NR, lsl])

        for c in range(PREF):
            issue_loads(c)

        # PE warm-up: HAM un-throttles after ~3.4us of sustained busy;
        # burn the preamble-to-first-chunk gap on the memset tile.
        # Dummies cycle through ps0p (shape/tag-matched).
        def dummy_mm():
            psd = ps0p.tile([HALF, TB2], FP32, tag="ps0")
            nc.tensor.matmul(psd[:, 0:TB], warm[:, 0:HALF], warm,
                             start=True, stop=True)

        for _ in range(N_DUMMY):
            dummy_mm()

        n_chunks = len(CHUNKS)
        for c, (xh0, xh1, col, csz) in enumerate(xts):
            tsl = slice(col, col + csz)
            o0_t = o0p.tile([NH, csz], BF16, tag="o0")
            o1_t = o1p.tile([NH, csz], BF16, tag="o1")
            for s in range((csz + TB2 - 1) // TB2):
                g0 = s * TB2
                gw = min(TB2, csz - g0)
                g = slice(g0, g0 + gw)
                # [104, 1024] PSUM tiles (2 banks); each matmul fills one
                ps0 = ps0p.tile([HALF, TB2], FP32, tag="ps0")
                ps1 = ps1p.tile([HALF, TB2], FP32, tag="ps1")
                for q0 in range(0, gw, TB):
                    qs = slice(g0 + q0, g0 + q0 + TB)
                    nc.tensor.matmul(ps0[:, q0 : q0 + TB], wh[:, 0:HALF],
                                     xh0[:, qs], start=True, stop=True)
                    nc.tensor.matmul(ps1[:, q0 : q0 + TB], wh[:, HALF:N],
                                     xh1[:, qs], start=True, stop=True)
                # evictions split across engines: block0 on VectorE,
                # block1 on ScalarE; both fuse the bias and fp32->bf16
                nc.vector.tensor_scalar_add(o0_t[0:HALF, g], ps0[:, 0:gw], bAc)
                nc.scalar.activation(o1_t[0:HALF, g], ps1[:, 0:gw], IDENT, bias=bBc)
            # HAM keep-alive: one dummy matmul fills the chunk-boundary
            # PE gap so the activity window never reads idle.
            if c + 1 < n_chunks:
                dummy_mm()
            # stores interleave with loads on the two HWDGE rings: o0
            # on Sync after the vector eviction, o1 on Scalar after the
            # scalar eviction; then each engine issues its chunk
            # c+PREF load. 112-row DMAs (partition count mult of 16).
            nc.sync.dma_start(o_d[0:NH, tsl], o0_t)
            nc.scalar.dma_start(o_d[NH : 2 * NH, tsl], o1_t)
            if c + PREF < n_chunks:
                issue_loads(c + PREF)


def _build():
    nc = bacc.Bacc(
        "TRN2",
        target_bir_lowering=False,
        debug=False,
        num_devices=N_CORES,
    )
    with tile.TileContext(nc) as tc:
        _kernel_body(tc)
    nc.compile()
    return nc


def kernel(x, W, b, S):
    global LAST_RESULTS
    nc = _CACHE.get("nc")
    if nc is None:
        nc = _build()
        _CACHE["nc"] = nc

    xf = np.asarray(x, np.float32).reshape(ROWS_TOTAL, N)
    SW = (np.asarray(S, np.float32) * np.asarray(W, np.float32))
    wh = np.zeros((NH, WPAD), NP_BF16)
    wh[:, 0:HALF] = SW[ROWS0, 0:HALF]
    wh[:, HALF:N] = SW[ROWS1, HALF:N]
    bfv = np.asarray(b, np.float32).reshape(N)
    bf = np.zeros((NH, BPAD), np.float32)
    bf[0:HALF, 0] = bfv[0:HALF]
    bf[0:HALF, 1] = bfv[HALF:N]

    in_maps = []
    for i in range(N_CORES):
        xt = xf[i * SHARD : (i + 1) * SHARD].T          # [208, SHARD] view
        xh = np.empty((NR, SHARD), NP_BF16)
        xh[0:K] = xt[N - K : N]
        xh[K : N + K] = xt
        xh[N + K : NR] = xt[0:K]
        in_maps.append({"xh": xh, "wh": wh, "bias": bf})
    res = run_bass_kernel_spmd(nc, in_maps, core_ids=list(range(N_CORES)))
    LAST_RESULTS = res
    out = np.empty((ROWS_TOTAL, N), np.float32)
    for i, r in enumerate(res.results):
        yt = r["outt"]                                  # [224, SHARD] bf16
        out[i * SHARD : (i + 1) * SHARD, 0:HALF] = yt[0:HALF].T
        out[i * SHARD : (i + 1) * SHARD, HALF:N] = yt[NH : NH + HALF].T
    return out.reshape(B, T, N)
